# revision 10
# baseline (speedup 1.0000x reference)
"""Actor-critic LSTM (masked recurrence) on 8 TRN2 NeuronCores.

Data-parallel: 32 sequences per core. Device-side everything runs in a
"transposed" layout (feature dim on SBUF partitions, batch on the free dim)
so the per-step LSTM elementwise work uses all 128 lanes and the recurrent
matmul needs no per-step transposes:

  gates^T[G, b] = sum_j W_cat^T-tiles (stationary) @ [x_t^T | h_t^T] (moving)

Per step (B=32 local seqs, H=256, 4H=1024):
  - 8 xp matmuls (W_ih^T g-tile stationary, x_t^T moving, N=32, start=True)
  - 16 rec matmuls (W_hh^T tiles, h_masked^T moving, accumulate)
  - DVE adds bias, ACT does sigmoid/tanh, DVE cell update, masks folded
    off the critical path (mask for step t+1 applied at the tail of step t).

Host does all free work: weight transposes/reorder to [f,i,o,g], bf16 casts,
h0/c0 pre-mask + transpose, mask table (1-starts)^T padded, and the inverse
transposes on outputs (logits/value come back as one fused (33, T*B) head^T).
"""

import os
import numpy as np
import ml_dtypes

import concourse.bass as bass
from concourse import bacc
import concourse.mybir as mybir
import concourse.tile as tile
from concourse.bass_utils import run_bass_kernel_spmd
from concourse.masks import make_identity

N_SEQ, T_FULL, IN, H, A = 256, 512, 128, 256, 32
NCORES = 8
B = N_SEQ // NCORES            # 32 sequences per core
G4 = 4 * H                     # 1024 gate rows
NG = G4 // 128                 # 8 gate tiles of 128
KH = H // 128                  # 2 k-tiles for the hidden contraction
T = int(os.environ.get("ACLSTM_T", T_FULL))   # debug knob
CH = min(128, T)               # chunk length (steps)
NCHUNK = T // CH
HEADP = 33                     # [W_logit; W_value] rows

f32 = mybir.dt.float32
bf16 = mybir.dt.bfloat16
AF = mybir.ActivationFunctionType


def _build_nc():
    nc = bacc.Bacc("TRN2", target_bir_lowering=False)

    feats = nc.dram_tensor("feats", [B * T, IN], f32, kind="ExternalInput")
    wih = nc.dram_tensor("wih", [IN, G4], bf16, kind="ExternalInput")       # W_ih_r^T
    whh = nc.dram_tensor("whh", [H, G4], bf16, kind="ExternalInput")        # W_hh_r^T
    whead = nc.dram_tensor("whead", [H, 64], bf16, kind="ExternalInput")    # [Wlogit;Wvalue]^T padded
    biasb = nc.dram_tensor("biasb", [128, NG, B], f32, kind="ExternalInput")  # gate bias bcast
    bh = nc.dram_tensor("bh", [64, 1], f32, kind="ExternalInput")           # head bias
    mT = nc.dram_tensor("mT", [T + 1, B], f32, kind="ExternalInput")        # 1-starts, padded
    h0T = nc.dram_tensor("h0T", [128, KH, B], bf16, kind="ExternalInput")   # pre-masked
    c0T = nc.dram_tensor("c0T", [128, KH, B], f32, kind="ExternalInput")

    headT = nc.dram_tensor("headT", [HEADP, T * B], f32, kind="ExternalOutput")
    hTo = nc.dram_tensor("hTo", [128, KH * B], f32, kind="ExternalOutput")
    cTo = nc.dram_tensor("cTo", [128, KH * B], f32, kind="ExternalOutput")
    DBG = os.environ.get("ACLSTM_DBG", "0") == "1"
    if DBG:
        dbg_g = nc.dram_tensor("dbg_g", [128, NG * B], f32, kind="ExternalOutput")
        dbg_h = nc.dram_tensor("dbg_h", [128, KH * B], f32, kind="ExternalOutput")
        dbg_c = nc.dram_tensor("dbg_c", [128, KH * B], f32, kind="ExternalOutput")
        dbg_x = nc.dram_tensor("dbg_x", [128, B], f32, kind="ExternalOutput")

    with tile.TileContext(nc) as tc:
        with (
            tc.tile_pool(name="const", bufs=1) as cpool,
            tc.tile_pool(name="xio", bufs=3) as xpool,
            tc.tile_pool(name="chunk", bufs=2) as kpool,
            tc.tile_pool(name="step", bufs=2) as spool,
            tc.tile_pool(name="carry", bufs=1) as carry,
            tc.tile_pool(name="psum", bufs=2, space="PSUM") as pp,
            tc.tile_pool(name="psumx", bufs=2, space="PSUM") as ppx,
        ):
            # ---- constants / weights ----
            # Everything the steady-state loop reads is staged through a DVE
            # copy: downstream instructions then depend on the single DVE
            # proc instead of many DMA queues / gpsimd, keeping per-
            # instruction semaphore wait counts under the HW-decode limit.
            def staged(shape, dt, tag, src_ap):
                stg = cpool.tile(shape, dt, tag=tag + "_stg")
                nc.sync.dma_start(stg, src_ap)
                dst = cpool.tile(shape, dt, tag=tag)
                nc.vector.tensor_copy(dst, stg)
                return dst

            ident_g = cpool.tile([128, 128], f32, tag="ident_g")
            make_identity(nc, ident_g)
            ident = cpool.tile([128, 128], f32, tag="ident")
            nc.vector.tensor_copy(ident, ident_g)

            wih_sb = staged([128, G4], bf16, "wih", wih[:, :])
            whh_sb = staged([128, KH, G4], bf16, "whh",
                            whh.rearrange("(j p) g -> p j g", p=128))
            whead_sb = staged([128, KH, 64], bf16, "whead",
                              whead.rearrange("(j p) m -> p j m", p=128))
            bias_sb = staged([128, NG, B], f32, "bias", biasb[:, :, :])
            bh_sb = staged([64, 1], f32, "bh", bh[:, :])

            # ---- carries ----
            hm_s = cpool.tile([128, KH, B], bf16, tag="hm_stg")
            nc.sync.dma_start(hm_s, h0T[:, :, :])
            hm = carry.tile([128, KH, B], bf16, tag="hm")      # masked h (rec input)
            nc.vector.tensor_copy(hm, hm_s)
            cst_s = cpool.tile([128, KH, B], f32, tag="cst_stg")
            nc.sync.dma_start(cst_s, c0T[:, :, :])
            cst = carry.tile([128, KH, B], f32, tag="cst")     # masked c
            nc.vector.tensor_copy(cst, cst_s)

            h_fin = carry.tile([128, KH, B], f32, tag="hfin")  # fp32 final h

            for k in range(NCHUNK):
                t0 = k * CH

                # ---- mask chunk: broadcast (CH+1, B) to all 128 partitions ----
                m_sb = kpool.tile([128, CH + 1, B], f32, tag="mask")
                nc.sync.dma_start(
                    m_sb, mT[t0:t0 + CH + 1, :].partition_broadcast(128)
                )

                # ---- x^T chunk: DMA natural (t,IN) tiles per seq, PE-transpose ----
                xT = kpool.tile([128, B, CH], bf16, tag="xT")
                for b in range(B):
                    xnat = xpool.tile([CH, 128], f32, tag="xnat")
                    nc.sync.dma_start(xnat, feats[b * T + t0: b * T + t0 + CH, :])
                    pxt = ppx.tile([128, CH], f32, tag="pxt")
                    nc.tensor.transpose(pxt, xnat, ident[:CH, :CH])
                    nc.vector.tensor_copy(xT[:, b, :], pxt)

                # ---- h history for the head ----
                hh = kpool.tile([128, KH, CH, B], bf16, tag="hh")

                for tl in range(CH):
                    # gates psum: [128, NG, B] accumulated by 24 matmuls
                    pg = pp.tile([128, NG, B], f32, tag="gates")
                    for g in range(NG):
                        # start=True clears the WHOLE psum bank -> only on the
                        # first matmul; later first-writes to a fresh region
                        # overwrite (has_written still clear) as needed.
                        nc.tensor.matmul(
                            pg[:, g, :], wih_sb[:, g * 128:(g + 1) * 128],
                            xT[:, :, tl], start=(g == 0), stop=False,
                        )
                    for g in range(NG):
                        for j in range(KH):
                            nc.tensor.matmul(
                                pg[:, g, :], whh_sb[:, j, g * 128:(g + 1) * 128],
                                hm[:, j, :], start=False, stop=(j == KH - 1),
                            )

                    gsb = spool.tile([128, NG, B], f32, tag="gsb")
                    nc.vector.tensor_add(gsb, pg, bias_sb)
                    if DBG and k == 0 and tl == 0:
                        nc.sync.dma_start(dbg_g[:, :],
                                          gsb.rearrange("p g b -> p (g b)"))
                        xdbg = spool.tile([128, B], f32, tag="xdbg")
                        nc.vector.tensor_copy(xdbg, xT[:, :, tl])
                        nc.sync.dma_start(dbg_x[:, :], xdbg)

                    # gate order [f,i,o,g]: tiles 0,1=f 2,3=i 4,5=o 6,7=g
                    sfio = spool.tile([128, 6, B], f32, tag="sfio")
                    nc.scalar.activation(sfio, gsb[:, 0:6, :], AF.Sigmoid)
                    tg = spool.tile([128, KH, B], f32, tag="tg")
                    nc.scalar.activation(tg, gsb[:, 6:8, :], AF.Tanh)

                    t1 = spool.tile([128, KH, B], f32, tag="t1")
                    nc.vector.tensor_mul(t1, sfio[:, 0:2, :], cst)
                    t2 = spool.tile([128, KH, B], f32, tag="t2")
                    nc.vector.tensor_mul(t2, sfio[:, 2:4, :], tg)
                    craw = spool.tile([128, KH, B], f32, tag="craw")
                    nc.vector.tensor_add(craw, t1, t2)

                    tcn = spool.tile([128, KH, B], f32, tag="tcn")
                    nc.scalar.activation(tcn, craw, AF.Tanh)

                    som = spool.tile([128, KH, B], f32, tag="som")
                    last = (k == NCHUNK - 1 and tl == CH - 1)
                    for j in range(KH):
                        # sigma_o * m_{t+1}: off the tanh_c critical path
                        nc.vector.tensor_mul(som[:, j, :], sfio[:, 4 + j, :],
                                             m_sb[:, tl + 1, :])
                    for j in range(KH):
                        # masked h for the recurrence
                        nc.vector.tensor_mul(hm[:, j, :], som[:, j, :], tcn[:, j, :])
                    for j in range(KH):
                        # unmasked h for the head
                        nc.vector.tensor_mul(hh[:, j, tl, :], sfio[:, 4 + j, :],
                                             tcn[:, j, :])
                    for j in range(KH):
                        # masked c for the next step (m padded with 1s at t=T)
                        nc.vector.tensor_mul(cst[:, j, :], craw[:, j, :],
                                             m_sb[:, tl + 1, :])
                    if last:
                        for j in range(KH):
                            nc.vector.tensor_mul(h_fin[:, j, :], sfio[:, 4 + j, :],
                                                 tcn[:, j, :])
                    if DBG and k == 0 and tl == 0:
                        hdbg = spool.tile([128, KH, B], f32, tag="hdbg")
                        nc.vector.tensor_copy(hdbg, hm)
                        nc.sync.dma_start(dbg_h[:, :],
                                          hdbg.rearrange("p j b -> p (j b)"))
                        nc.sync.dma_start(dbg_c[:, :],
                                          cst.rearrange("p j b -> p (j b)"))

                # ---- head: [33, 512]-tile matmuls over the chunk's h history ----
                hhf = hh.rearrange("p j t b -> p j (t b)")
                for off in range(0, CH * B, 512):
                    w = min(512, CH * B - off)
                    ph = pp.tile([HEADP, 512], f32, tag="ph")
                    for j in range(KH):
                        nc.tensor.matmul(
                            ph[:, :w], whead_sb[:, j, :HEADP],
                            hhf[:, j, off:off + w],
                            start=(j == 0), stop=(j == KH - 1),
                        )
                    hd = spool.tile([HEADP, 512], f32, tag="hd")
                    nc.scalar.activation(hd[:, :w], ph[:, :w], AF.Identity,
                                         bias=bh_sb[:HEADP, :])
                    nc.sync.dma_start(
                        headT[:, t0 * B + off: t0 * B + off + w], hd[:, :w]
                    )

            nc.sync.dma_start(hTo[:, :], h_fin.rearrange("p j b -> p (j b)"))
            nc.sync.dma_start(cTo[:, :], cst.rearrange("p j b -> p (j b)"))

    nc.finalize()
    return nc


_NC_CACHE = {}


def _get_nc():
    if "nc" not in _NC_CACHE:
        _NC_CACHE["nc"] = _build_nc()
    return _NC_CACHE["nc"]


# gate reorder: torch order [i,f,g,o] -> device order [f,i,o,g]
_PERM = np.r_[H:2 * H, 0:H, 3 * H:4 * H, 2 * H:3 * H]


def _prep_shared(W_ih, W_hh, b_ih, b_hh, W_logit, b_logit, W_value, b_value):
    wih_r = W_ih[_PERM]                       # (1024, 128)
    whh_r = W_hh[_PERM]                       # (1024, 256)
    b_r = (b_ih + b_hh)[_PERM]                # (1024,)

    wih_t = np.ascontiguousarray(wih_r.T).astype(ml_dtypes.bfloat16)   # (128,1024)
    whh_t = np.ascontiguousarray(whh_r.T).astype(ml_dtypes.bfloat16)   # (256,1024)

    whd = np.zeros((64, H), np.float32)
    whd[:A] = W_logit
    whd[A] = W_value[0]
    whd_t = np.ascontiguousarray(whd.T).astype(ml_dtypes.bfloat16)     # (256, 64)

    bh = np.zeros((64, 1), np.float32)
    bh[:A, 0] = b_logit
    bh[A, 0] = b_value[0]

    # bias broadcast tile (128, 8, 32): partition=G%128, free=(gtile, b)
    bias_b = np.ascontiguousarray(
        np.broadcast_to(b_r.reshape(NG, 128).T[:, :, None], (128, NG, B))
    ).astype(np.float32)

    return wih_t, whh_t, whd_t, bh, bias_b


def _run(inputs, trace=False):
    (features, episode_starts, h0, c0, W_ih, W_hh, b_ih, b_hh,
     W_logit, b_logit, W_value, b_value) = (
        inputs["features"], inputs["episode_starts"], inputs["h0"], inputs["c0"],
        inputs["W_ih"], inputs["W_hh"], inputs["b_ih"], inputs["b_hh"],
        inputs["W_logit"], inputs["b_logit"], inputs["W_value"], inputs["b_value"])

    features = np.asarray(features, np.float32)
    episode_starts = np.asarray(episode_starts, np.float32)
    h0 = np.asarray(h0, np.float32)
    c0 = np.asarray(c0, np.float32)

    wih_t, whh_t, whd_t, bh, bias_b = _prep_shared(
        np.asarray(W_ih, np.float32), np.asarray(W_hh, np.float32),
        np.asarray(b_ih, np.float32), np.asarray(b_hh, np.float32),
        np.asarray(W_logit, np.float32), np.asarray(b_logit, np.float32),
        np.asarray(W_value, np.float32), np.asarray(b_value, np.float32))

    featr = features.reshape(N_SEQ, T_FULL, IN)[:, :T]
    startr = episode_starts.reshape(N_SEQ, T_FULL)[:, :T]

    in_maps = []
    for i in range(NCORES):
        sl = slice(i * B, (i + 1) * B)
        fshard = np.ascontiguousarray(featr[sl].reshape(B * T, IN))
        s = startr[sl]                                    # (B, T)
        m = np.ones((T + 1, B), np.float32)
        m[:T] = (1.0 - s).T
        m0 = m[0][:, None]                                # (B, 1)
        h0s = (h0[0, sl] * m0).T.reshape(KH, 128, B).transpose(1, 0, 2)
        c0s = (c0[0, sl] * m0).T.reshape(KH, 128, B).transpose(1, 0, 2)
        in_maps.append(dict(
            feats=fshard, mT=m,
            h0T=np.ascontiguousarray(h0s).astype(ml_dtypes.bfloat16),
            c0T=np.ascontiguousarray(c0s).astype(np.float32),
            wih=wih_t, whh=whh_t, whead=whd_t, bh=bh, biasb=bias_b,
        ))

    nc = _get_nc()
    res = run_bass_kernel_spmd(nc, in_maps, core_ids=list(range(NCORES)),
                               trace=trace)

    logits = np.empty((N_SEQ * T, A), np.float32)
    value = np.empty((N_SEQ * T, 1), np.float32)
    h_out = np.empty((1, N_SEQ, H), np.float32)
    c_out = np.empty((1, N_SEQ, H), np.float32)
    for i in range(NCORES):
        r = res.results[i]
        ht = r["headT"]                                   # (33, T*B) cols t*B+b
        lg = ht[:A].T.reshape(T, B, A).transpose(1, 0, 2) # (B, T, A)
        logits[i * B * T:(i + 1) * B * T] = lg.reshape(B * T, A)
        vv = ht[A].reshape(T, B).T                        # (B, T)
        value[i * B * T:(i + 1) * B * T, 0] = vv.reshape(B * T)
        hT = r["hTo"].reshape(128, KH, B)                 # [p, j, b]
        cT = r["cTo"].reshape(128, KH, B)
        h_out[0, i * B:(i + 1) * B] = hT.transpose(2, 1, 0).reshape(B, H)
        c_out[0, i * B:(i + 1) * B] = cT.transpose(2, 1, 0).reshape(B, H)

    return (logits, value, h_out, c_out), res


def kernel(**inputs):
    out, _ = _run(inputs, trace=False)
    return out
